# revision 24
# baseline (speedup 1.0000x reference)
"""GsLmkEncoder Trainium2 kernel.

out[n, b*68+k] = enc_b(n,k) * exp(-0.5 * wq(n,k)),   b in 0..4
  enc_0 = dz = (x_n - l_k) . rz
  enc_1 = sin(dz), enc_2 = cos(dz), enc_3 = sin(2 dz), enc_4 = cos(2 dz)
  wq = (x_n - l_k)^T cov_k (x_n - l_k)

Reformulation: with s_n = x_n . rz and t_k = l_k . rz, dz = s_n - t_k, so
sin/cos(dz) and sin/cos(2 dz) expand by angle addition into products of
per-point trig and per-landmark trig; wq and dz are quadratic/linear in x.
Everything factors as ONE bf16 matmul F[n, 32] @ G[32, 408] per 128-point
tile, where the wq part survives catastrophic cancellation (terms ~1/sigma^2
~ 400 cancel to O(1)) through an error-compensated split: bf16 products are
exact in the f32 PSUM accumulator, so splitting both the quadratic features
and the G entries into bf16 hi/lo parts (F*G ~ Fh*Gh + Fl*Gh + Fh*Gl)
recovers ~f32 accuracy at full bf16 PE speed. The f32 residual of the
per-landmark constant c = l^T cov l has no feature row left (32-row budget);
its low part cl folds into the output as exp(-0.5*cl_k), a bounded
per-landmark factor scaled into G's five enc blocks on the host.

F rows (feature stride 32 so each tile's F^T is 32-aligned for PE row
groups 0/32/64): [qh(6) | ql(6) | qh(6) | xh(3) | xl(3) | xh(3) | 1 |
sinS cosS sin2S cos2S], where q = [x0^2,x1^2,x2^2,x0x1,x1x2,x2x0],
qh = bf16(q) (the f32->bf16 tensor_copy performs the split on-device so it
is bitwise consistent with what the PE sees), ql = q - qh.

Device schedule per shard (25344 points = 66 groups of 3 tiles of 128):
 - prologue in 2 chunks (24 + 174 tiles) so the pipeline starts early:
   s = x . rz, range-reduce, ACT sins; F assembled by bulk strided ops
   (DVE for the f32 quad chain, Pool for SBUF-only copies) into
   F_all [128, 198, 32] bf16.
 - per 3-tile group: one bf16 PE transpose [128, 96] -> F^T at row groups
   0/32/64 (+ ACT copy to SBUF); 3 bf16 matmuls [K=32 -> 408 cols] ->
   f32 PSUM [128, 3, 512]; one batched ACT exp -> w; one DVE multiply
   (w broadcast over the 5 enc blocks) writing bf16; contiguous DMA out.
 - bf16 output halves HBM write traffic; host converts back to f32.
   The DVE multiply reads PSUM at the 1x tensor_tensor rate (the 2x/4x
   DVE uops require all-SBUF 2-byte operands), which at ~87us is the
   critical engine; PE/ACT/DMA/Pool all run in its shadow.
"""

import sys
import numpy as np

for _p in ("/opt/trn_rl_repo", "/root/.axon_site/_ro/pypackages"):
    if _p not in sys.path:
        sys.path.insert(0, _p)

import concourse.bass as bass
import concourse.bacc as bacc
import concourse.tile as tile
from concourse import mybir
from concourse.masks import make_identity
from concourse.bass_utils import run_bass_kernel_spmd

# Wire the NTFF profile hook (the agent image's antenv lacks axon_hooks);
# without it trace=True silently degrades to no profiling.
try:
    import antenv.axon_hooks  # noqa: F401
except ImportError:
    try:
        import types as _types

        sys.path.insert(0, "/root/.axon_site")
        from trn_agent_boot.trn_boot import _ntff_profile_via_ctypes

        _hook = _ntff_profile_via_ctypes("/opt/axon/libaxon_pjrt.so")
        _m = _types.ModuleType("antenv.axon_hooks")
        _m.get_axon_ntff_profile_hook = lambda: _hook
        _m.set_axon_ntff_profile_hook = lambda h: None
        sys.modules["antenv.axon_hooks"] = _m
    except Exception:
        pass

F32 = mybir.dt.float32
F16 = mybir.dt.float16
BF16 = mybir.dt.bfloat16
AF = mybir.ActivationFunctionType
OP = mybir.AluOpType

N = 200000
L = 68
OUT_DIM = 5 * L  # 340
NCOLS = 6 * L    # 408: [wq | dz | s1 | c1 | s2 | c2]
K = 32           # feature rows (full 32-row group)
NCORES = 8
NTILES = 198                 # tiles of 128 points per shard (3 | 198)
NPAD = NTILES * 128          # 25344 per shard
GRP = 3                      # tiles per transpose / psum / out group
NGRP = NTILES // GRP         # 66
KS = 32                      # feature partition stride (PE base-partition rule)
HALF_PI = float(np.pi / 2)
TWO_PI = float(np.float32(2 * np.pi))
PI_F = float(np.float32(np.pi))
INV_2PI = float(np.float32(1.0 / (2 * np.pi)))
INV_PI = float(np.float32(1.0 / np.pi))
MAGIC = 12582912.0  # 1.5 * 2**23: add+sub rounds f32 to nearest int
# clamp bounds keeping func(scale*in+bias) strictly inside [-pi, pi]
B1 = 3.141590
C1 = (-B1, B1)
C1C = (-(B1 + HALF_PI), float(np.float32(B1 - HALF_PI)))
C2 = (-B1 / 2, B1 / 2)
C2C = (-(B1 + HALF_PI) / 2, float(np.float32((B1 - HALF_PI) / 2)))

POOL_MULT = False  # Pool cannot read PSUM (BIR verifier)


def _bcast_block(ap, nrep, block):
    """Insert a stride-0 dim of size nrep before the last dim (size block)."""
    new = ap.copy()
    pat = [list(d) for d in new.ap]
    assert pat[-1][1] == block, (pat, block)
    pat.insert(len(pat) - 1, [0, nrep])
    return bass.AP(ap.tensor, ap.offset, pat)


def bf16_split(a):
    """Round-to-nearest-even bf16 high part (as f64) and residual."""
    a32 = np.asarray(a, np.float32)
    u = a32.view(np.uint32)
    hi = ((u + 0x7FFF + ((u >> 16) & 1)) & 0xFFFF0000).astype(np.uint32)
    h = hi.view(np.float32).astype(np.float64)
    return h, np.asarray(a, np.float64) - h


def build_nc(ntiles=NTILES):
    npad = ntiles * 128
    ngrp = ntiles // GRP
    assert ngrp * GRP == ntiles
    nc = bacc.Bacc("TRN2", target_bir_lowering=False, debug=False, num_devices=NCORES)
    x_d = nc.dram_tensor("x", [npad, 3], F32, kind="ExternalInput")
    g_d = nc.dram_tensor("g", [128, NCOLS], F32, kind="ExternalInput")
    rz_d = nc.dram_tensor("rzb", [128, 3], F32, kind="ExternalInput")
    out_d = nc.dram_tensor("out", [npad, OUT_DIM], BF16, kind="ExternalOutput")

    with tile.TileContext(nc) as tc:
        with (
            tc.tile_pool(name="const", bufs=1) as constp,
            tc.tile_pool(name="wpool", bufs=4) as wpool,
            tc.tile_pool(name="opool", bufs=4) as opool,
            tc.tile_pool(name="c2p", bufs=3) as c2pool,
            tc.tile_pool(name="mmps", bufs=2, space="PSUM") as mmpsp,
            tc.tile_pool(name="ftps", bufs=2, space="PSUM") as ftpsp,
        ):
            # ---- persistent tiles ----
            x_sb = constp.tile([128, ntiles, 3], F32)       # grouped point layout
            xb_sb = constp.tile([128, ntiles, 3], F32)      # rotated [x1,x2,x0]
            s_all = constp.tile([128, ntiles], F32)
            ang = constp.tile([128, ntiles, 4], F32)
            scr = constp.tile([128, ntiles], F32)
            trig = constp.tile([128, ntiles, 4], F32)
            q32 = constp.tile([128, ntiles, 6], F32)
            qh32 = constp.tile([128, ntiles, 6], F32)
            xh32 = constp.tile([128, ntiles, 3], F32)
            f_all = constp.tile([128, ntiles, KS], BF16)
            ft_all = constp.tile([128, ngrp, 128], BF16)
            g32_sb = constp.tile([128, NCOLS], F32)
            g_sb = constp.tile([128, NCOLS], BF16)
            rz_sb = constp.tile([128, 3], F32)
            ident = constp.tile([128, 128], F32)
            ident_bf = constp.tile([128, 128], BF16)

            nc.sync.dma_start(rz_sb[:], rz_d[:])
            make_identity(nc, ident[:])
            nc.gpsimd.tensor_copy(ident_bf[:], ident[:])
            bias_hpi = constp.tile([128, 1], F32)
            nc.gpsimd.memset(bias_hpi[:], HALF_PI)
            nc.gpsimd.memset(f_all[:, :, 27:28], 1.0)  # const feature
            ncf = GRP * KS  # 96 transposed columns per group

            # x load: partition p holds points p*ntiles .. p*ntiles+ntiles-1;
            # first 24 tiles land in their own DMA so chunk 0 starts early
            x_r = x_d[:].rearrange("(p m) c -> p m c", p=128)
            nc.sync.dma_start(x_sb[:, 0:24], x_r[:, 0:24])
            nc.sync.dma_start(g32_sb[:], g_d[:])
            nc.gpsimd.tensor_copy(g_sb[:], g32_sb[:])
            nc.sync.dma_start(x_sb[:, 24:ntiles], x_r[:, 24:ntiles])

            # ---- prologue (chunked so the main loop can start early) ----
            chunk_bounds = [0, 24, 108, ntiles]  # small first chunk: fast start
            fams = [
                (INV_2PI, 0.0, -TWO_PI, C1),     # sin(s)
                (INV_2PI, 0.25, -TWO_PI, C1C),   # sin(s + pi/2)
                (INV_PI, 0.0, -PI_F, C2),        # sin(2s)
                (INV_PI, 0.25, -PI_F, C2C),      # sin(2s + pi/2)
            ]
            def emit_prologue(cc):
                t0, t1 = chunk_bounds[cc], chunk_bounds[cc + 1]
                # s = x . rz  (tensor_scalar: scalar slot is per-partition)
                if cc == 0:
                    nc.vector.tensor_tensor(
                        scr[:, 0:3], x_sb[:, 0, :], rz_sb[:, 0:3], OP.mult
                    )
                nc.vector.tensor_scalar(
                    s_all[:, t0:t1], x_sb[:, t0:t1, 0], rz_sb[:, 0:1],
                    None, OP.mult
                )
                nc.vector.scalar_tensor_tensor(
                    s_all[:, t0:t1], x_sb[:, t0:t1, 1], rz_sb[:, 1:2],
                    s_all[:, t0:t1], OP.mult, OP.add
                )
                nc.vector.scalar_tensor_tensor(
                    s_all[:, t0:t1], x_sb[:, t0:t1, 2], rz_sb[:, 2:3],
                    s_all[:, t0:t1], OP.mult, OP.add
                )
                # range-reduce the four angle families into [-pi, pi] after
                # the activation's own scale/bias is applied; n = round(...)
                # via the 1.5*2^23 magic constant (delta added pre-magic)
                for ci, (inv, delta, mul, (lo, hi)) in enumerate(fams):
                    nc.vector.tensor_scalar(
                        scr[:, t0:t1], s_all[:, t0:t1], inv, delta,
                        OP.mult, OP.add
                    )
                    nc.vector.tensor_scalar(
                        scr[:, t0:t1], scr[:, t0:t1], MAGIC, MAGIC,
                        OP.add, OP.subtract
                    )
                    nc.vector.scalar_tensor_tensor(
                        scr[:, t0:t1], scr[:, t0:t1], mul, s_all[:, t0:t1],
                        OP.mult, OP.add
                    )
                    nc.vector.tensor_scalar(
                        ang[:, t0:t1, ci], scr[:, t0:t1], hi, lo,
                        OP.min, OP.max
                    )
                # ---- F assembly for this chunk ----
                nc.vector.tensor_copy(
                    xb_sb[:, t0:t1, 0:2], x_sb[:, t0:t1, 1:3]
                )
                nc.vector.tensor_copy(
                    xb_sb[:, t0:t1, 2:3], x_sb[:, t0:t1, 0:1]
                )
                nc.vector.tensor_tensor(
                    q32[:, t0:t1, :3], x_sb[:, t0:t1], x_sb[:, t0:t1], OP.mult
                )
                nc.vector.tensor_tensor(
                    q32[:, t0:t1, 3:], x_sb[:, t0:t1], xb_sb[:, t0:t1], OP.mult
                )
                # qh = bf16(q) happens in the f32->bf16 copy; ql = q - qh
                nc.vector.tensor_copy(f_all[:, t0:t1, 0:6], q32[:, t0:t1])
                nc.vector.tensor_copy(qh32[:, t0:t1], f_all[:, t0:t1, 0:6])
                nc.vector.tensor_tensor(
                    f_all[:, t0:t1, 6:12], q32[:, t0:t1], qh32[:, t0:t1],
                    OP.subtract
                )
                nc.gpsimd.tensor_copy(
                    f_all[:, t0:t1, 12:18], f_all[:, t0:t1, 0:6]
                )
                # x-chain + trig copy on the idle Pool engine (SBUF-only)
                nc.gpsimd.tensor_copy(f_all[:, t0:t1, 18:21], x_sb[:, t0:t1])
                nc.gpsimd.tensor_copy(xh32[:, t0:t1], f_all[:, t0:t1, 18:21])
                nc.gpsimd.tensor_tensor(
                    f_all[:, t0:t1, 21:24], x_sb[:, t0:t1], xh32[:, t0:t1],
                    OP.subtract
                )
                nc.gpsimd.tensor_copy(
                    f_all[:, t0:t1, 24:27], f_all[:, t0:t1, 18:21]
                )

            def emit_sins(cc):
                t0, t1 = chunk_bounds[cc], chunk_bounds[cc + 1]
                nc.scalar.activation(
                    trig[:, t0:t1, 0], ang[:, t0:t1, 0], AF.Sin
                )
                nc.scalar.activation(
                    trig[:, t0:t1, 1], ang[:, t0:t1, 1], AF.Sin,
                    bias=bias_hpi[:]
                )
                nc.scalar.activation(
                    trig[:, t0:t1, 2], ang[:, t0:t1, 2], AF.Sin, scale=2.0
                )
                nc.scalar.activation(
                    trig[:, t0:t1, 3], ang[:, t0:t1, 3], AF.Sin,
                    bias=bias_hpi[:], scale=2.0
                )
                nc.gpsimd.tensor_copy(f_all[:, t0:t1, 28:32], trig[:, t0:t1])

            # ---- main loop: per 3-tile group ----
            def emit_transpose(g):
                ft_ps = ftpsp.tile([128, 128], BF16, tag="FT")
                nc.tensor.matmul(
                    ft_ps[0:ncf, 0:128],
                    f_all[:, g * GRP : (g + 1) * GRP, :],
                    ident_bf[:],
                    is_transpose=True,
                )
                nc.scalar.copy(ft_all[0:ncf, g, :], ft_ps[0:ncf, :])

            emit_prologue(0)
            emit_sins(0)
            emit_prologue(1)
            emit_prologue(2)
            emit_transpose(0)
            for g in range(ngrp):
                if g + 1 < ngrp:
                    emit_transpose(g + 1)

                out_rows = out_d[:].rearrange("(p m) c -> p (m c)", p=128)[
                    :, g * GRP * OUT_DIM : (g + 1) * GRP * OUT_DIM
                ]
                o_t = opool.tile([128, GRP, 5, L], BF16, tag="O")
                psum = mmpsp.tile([128, GRP, 512], F32, tag="P")
                for j in range(GRP):
                    m = j * KS
                    nc.tensor.matmul(
                        psum[:, j, 0:NCOLS],
                        ft_all[m : m + K, g, :],
                        g_sb[m : m + K, :],
                        start=True,
                        stop=True,
                    )
                w_t = wpool.tile([128, GRP, L], BF16, tag="W")
                nc.scalar.activation(
                    w_t[:], psum[:, :, 0:L], AF.Exp, scale=-0.5
                )
                # stage the c2 block to SBUF bf16 on ACT (it has slack) so
                # its weighted multiply runs on the DVE 2-byte 2x fast path;
                # the other 4 blocks multiply straight from PSUM at 1x
                c2_sb = c2pool.tile([128, GRP, L], BF16, tag="C2")
                nc.scalar.copy(c2_sb[:], psum[:, :, 5 * L : NCOLS])
                enc4 = psum[:, :, L : 5 * L].rearrange(
                    "p t (b l) -> p t b l", l=L
                )
                wb4 = _bcast_block(w_t[:], 4, L)
                nc.vector.tensor_tensor(o_t[:, :, 0:4, :], enc4, wb4, OP.mult)
                nc.vector.tensor_tensor(
                    o_t[:, :, 4, :], c2_sb[:], w_t[:], OP.mult
                )
                nc.sync.dma_start(
                    out_rows[:], o_t[:].rearrange("p t b l -> p (t b l)")
                )
                if g == 0:
                    # ACT is in-order: chunk 1/2 sins go behind exp(0) so the
                    # first weighted multiplies are never blocked
                    emit_sins(1)
                    emit_sins(2)
    nc.compile()
    return nc


def host_params(l, r, scaling, rotation):
    """G [128, 408] f32 (bf16-representable, replicated at 3 row-group
    bases) + rz broadcast. Feature rows per 32-block:
      0-5   qh  * Gh(quad)      6-11  ql * Gh(quad)   12-17 qh * Gl(quad)
      18-20 xh  * bh | rz*eta   21-23 xl * bh | rz*eta
      24-26 xh  * bl            27    1  * ch | -t*eta
      28-31 trig * (per-landmark trig * eta)
    where eta_k = exp(-0.5 * cl_k) folds the bf16 residual of c into the
    five enc blocks."""
    l = l.astype(np.float64)
    r = r.astype(np.float64)
    scaling = scaling.astype(np.float64)
    rotation = rotation.astype(np.float64)

    rz = r[:3, 2]
    qn = rotation / np.maximum(
        np.linalg.norm(rotation, axis=1, keepdims=True), 1e-12
    )
    w, x, y, z = qn[:, 0], qn[:, 1], qn[:, 2], qn[:, 3]
    R = np.empty((L, 3, 3), np.float64)
    R[:, 0, 0] = 1 - 2 * (y * y + z * z)
    R[:, 0, 1] = 2 * (x * y - w * z)
    R[:, 0, 2] = 2 * (x * z + w * y)
    R[:, 1, 0] = 2 * (x * y + w * z)
    R[:, 1, 1] = 1 - 2 * (x * x + z * z)
    R[:, 1, 2] = 2 * (y * z - w * x)
    R[:, 2, 0] = 2 * (x * z - w * y)
    R[:, 2, 1] = 2 * (y * z + w * x)
    R[:, 2, 2] = 1 - 2 * (x * x + y * y)
    M = R / scaling[:, None, :]
    cov = np.einsum("lij,lkj->lik", M, M)       # [L,3,3]

    b = np.einsum("lij,lj->li", cov, l)         # cov_k @ l_k
    c = np.einsum("li,li->l", l, b)             # l^T cov l
    t = l @ rz

    # quad rows matching features [x0^2, x1^2, x2^2, x0x1, x1x2, x2x0]
    Gq = np.stack(
        [
            cov[:, 0, 0],
            cov[:, 1, 1],
            cov[:, 2, 2],
            2 * cov[:, 0, 1],
            2 * cov[:, 1, 2],
            2 * cov[:, 0, 2],
        ]
    )                                            # [6, L]
    Gq_h, Gq_l = bf16_split(Gq)
    b2 = -2.0 * b.T                              # [3, L]
    b2_h, b2_l = bf16_split(b2)
    c_h, c_l = bf16_split(c)
    eta = np.exp(-0.5 * c_l)                     # bounded: |c_l| <= |c|*2^-8

    c1, s1 = np.cos(t), np.sin(t)
    c2, s2 = np.cos(2 * t), np.sin(2 * t)

    G = np.zeros((K, NCOLS), np.float64)
    # wq block
    G[0:6, 0:L] = Gq_h
    G[6:12, 0:L] = Gq_h
    G[12:18, 0:L] = Gq_l
    G[18:21, 0:L] = b2_h
    G[21:24, 0:L] = b2_h
    G[24:27, 0:L] = b2_l
    G[27, 0:L] = c_h
    # dz block (xh + xl recovers full-precision x)
    G[18:21, L : 2 * L] = rz[:, None] * eta[None, :]
    G[21:24, L : 2 * L] = rz[:, None] * eta[None, :]
    G[27, L : 2 * L] = -t * eta
    # trig blocks: sin(s-t) = sinS cosT - cosS sinT ; cos(s-t) = ...
    G[28, 2 * L : 3 * L] = c1 * eta
    G[29, 2 * L : 3 * L] = -s1 * eta
    G[28, 3 * L : 4 * L] = s1 * eta
    G[29, 3 * L : 4 * L] = c1 * eta
    G[30, 4 * L : 5 * L] = c2 * eta
    G[31, 4 * L : 5 * L] = -s2 * eta
    G[30, 5 * L : 6 * L] = s2 * eta
    G[31, 5 * L : 6 * L] = c2 * eta
    # pre-round everything to bf16-representable f32 so the host splits
    # are exactly what the PE multiplies
    Gbf, _ = bf16_split(G)
    Grep = np.zeros((128, NCOLS), np.float32)
    for m in range(GRP):
        Grep[m * KS : m * KS + K, :] = Gbf.astype(np.float32)
    return Grep, np.broadcast_to(
        rz.astype(np.float32), (128, 3)
    ).copy()


_NC_CACHE = {}


def _get_nc():
    if "nc" not in _NC_CACHE:
        _NC_CACHE["nc"] = build_nc()
    return _NC_CACHE["nc"]


def run(inputs, mm_f32r=True, trace=False):
    x = inputs["x"]
    G, rzb = host_params(
        inputs["l"], inputs["r"], inputs["scaling"], inputs["rotation"]
    )
    xpad = np.zeros((NCORES * NPAD, 3), np.float32)
    xpad[:N] = x
    shards = xpad.reshape(NCORES, NPAD, 3)
    in_maps = [
        {"x": np.ascontiguousarray(shards[i]), "g": G, "rzb": rzb}
        for i in range(NCORES)
    ]
    nc = _get_nc()
    res = run_bass_kernel_spmd(nc, in_maps, list(range(NCORES)), trace=trace)
    out = np.concatenate([r["out"] for r in res.results], axis=0)[:N]
    return out.astype(np.float32), res


def kernel(**inputs):
    out, _ = run(inputs)
    return out


# revision 25
# speedup vs baseline: 1.0645x; 1.0645x over previous
"""GsLmkEncoder Trainium2 kernel.

out[n, b*68+k] = enc_b(n,k) * exp(-0.5 * wq(n,k)),   b in 0..4
  enc_0 = dz = (x_n - l_k) . rz
  enc_1 = sin(dz), enc_2 = cos(dz), enc_3 = sin(2 dz), enc_4 = cos(2 dz)
  wq = (x_n - l_k)^T cov_k (x_n - l_k)

Reformulation: with s_n = x_n . rz and t_k = l_k . rz, dz = s_n - t_k, so
sin/cos(dz) and sin/cos(2 dz) expand by angle addition into products of
per-point trig and per-landmark trig; wq and dz are quadratic/linear in x.
Everything factors as ONE bf16 matmul F[n, 32] @ G[32, 408] per 128-point
tile, where the wq part survives catastrophic cancellation (terms ~1/sigma^2
~ 400 cancel to O(1)) through an error-compensated split: bf16 products are
exact in the f32 PSUM accumulator, so splitting both the quadratic features
and the G entries into bf16 hi/lo parts (F*G ~ Fh*Gh + Fl*Gh + Fh*Gl)
recovers ~f32 accuracy at full bf16 PE speed. The f32 residual of the
per-landmark constant c = l^T cov l has no feature row left (32-row budget);
its low part cl folds into the output as exp(-0.5*cl_k), a bounded
per-landmark factor scaled into G's five enc blocks on the host.

F rows (feature stride 32 so each tile's F^T is 32-aligned for PE row
groups 0/32/64): [qh(6) | ql(6) | qh(6) | xh(3) | xl(3) | xh(3) | 1 |
sinS cosS sin2S cos2S], where q = [x0^2,x1^2,x2^2,x0x1,x1x2,x2x0],
qh = bf16(q) (the f32->bf16 tensor_copy performs the split on-device so it
is bitwise consistent with what the PE sees), ql = q - qh.

Device schedule per shard (25344 points = 66 groups of 3 tiles of 128):
 - prologue in 2 chunks (24 + 174 tiles) so the pipeline starts early:
   s = x . rz, range-reduce, ACT sins; F assembled by bulk strided ops
   (DVE for the f32 quad chain, Pool for SBUF-only copies) into
   F_all [128, 198, 32] bf16.
 - per 3-tile group: one bf16 PE transpose [128, 96] -> F^T at row groups
   0/32/64 (+ ACT copy to SBUF); 3 bf16 matmuls [K=32 -> 408 cols] ->
   f32 PSUM [128, 3, 512]; one batched ACT exp -> w; one DVE multiply
   (w broadcast over the 5 enc blocks) writing bf16; contiguous DMA out.
 - bf16 output halves HBM write traffic; host converts back to f32.
   The DVE multiply reads PSUM at the 1x tensor_tensor rate (the 2x/4x
   DVE uops require all-SBUF 2-byte operands), which at ~87us is the
   critical engine; PE/ACT/DMA/Pool all run in its shadow.
"""

import sys
import numpy as np

for _p in ("/opt/trn_rl_repo", "/root/.axon_site/_ro/pypackages"):
    if _p not in sys.path:
        sys.path.insert(0, _p)

import concourse.bass as bass
import concourse.bacc as bacc
import concourse.tile as tile
from concourse import mybir
from concourse.masks import make_identity
from concourse.bass_utils import run_bass_kernel_spmd

# Wire the NTFF profile hook (the agent image's antenv lacks axon_hooks);
# without it trace=True silently degrades to no profiling.
try:
    import antenv.axon_hooks  # noqa: F401
except ImportError:
    try:
        import types as _types

        sys.path.insert(0, "/root/.axon_site")
        from trn_agent_boot.trn_boot import _ntff_profile_via_ctypes

        _hook = _ntff_profile_via_ctypes("/opt/axon/libaxon_pjrt.so")
        _m = _types.ModuleType("antenv.axon_hooks")
        _m.get_axon_ntff_profile_hook = lambda: _hook
        _m.set_axon_ntff_profile_hook = lambda h: None
        sys.modules["antenv.axon_hooks"] = _m
    except Exception:
        pass

F32 = mybir.dt.float32
F16 = mybir.dt.float16
BF16 = mybir.dt.bfloat16
AF = mybir.ActivationFunctionType
OP = mybir.AluOpType

N = 200000
L = 68
OUT_DIM = 5 * L  # 340
NCOLS = 6 * L    # 408: [wq | dz | s1 | c1 | s2 | c2]
K = 32           # feature rows (full 32-row group)
NCORES = 8
NTILES = 198                 # tiles of 128 points per shard (3 | 198)
NPAD = NTILES * 128          # 25344 per shard
GRP = 3                      # tiles per transpose / psum / out group
NGRP = NTILES // GRP         # 66
KS = 32                      # feature partition stride (PE base-partition rule)
HALF_PI = float(np.pi / 2)
TWO_PI = float(np.float32(2 * np.pi))
PI_F = float(np.float32(np.pi))
INV_2PI = float(np.float32(1.0 / (2 * np.pi)))
INV_PI = float(np.float32(1.0 / np.pi))
MAGIC = 12582912.0  # 1.5 * 2**23: add+sub rounds f32 to nearest int
# clamp bounds keeping func(scale*in+bias) strictly inside [-pi, pi]
B1 = 3.141590
C1 = (-B1, B1)
C1C = (-(B1 + HALF_PI), float(np.float32(B1 - HALF_PI)))
C2 = (-B1 / 2, B1 / 2)
C2C = (-(B1 + HALF_PI) / 2, float(np.float32((B1 - HALF_PI) / 2)))

POOL_MULT = False  # Pool cannot read PSUM (BIR verifier)


def _bcast_block(ap, nrep, block):
    """Insert a stride-0 dim of size nrep before the last dim (size block)."""
    new = ap.copy()
    pat = [list(d) for d in new.ap]
    assert pat[-1][1] == block, (pat, block)
    pat.insert(len(pat) - 1, [0, nrep])
    return bass.AP(ap.tensor, ap.offset, pat)


def bf16_split(a):
    """Round-to-nearest-even bf16 high part (as f64) and residual."""
    a32 = np.asarray(a, np.float32)
    u = a32.view(np.uint32)
    hi = ((u + 0x7FFF + ((u >> 16) & 1)) & 0xFFFF0000).astype(np.uint32)
    h = hi.view(np.float32).astype(np.float64)
    return h, np.asarray(a, np.float64) - h


def build_nc(ntiles=NTILES):
    npad = ntiles * 128
    ngrp = ntiles // GRP
    assert ngrp * GRP == ntiles
    nc = bacc.Bacc("TRN2", target_bir_lowering=False, debug=False, num_devices=NCORES)
    x_d = nc.dram_tensor("x", [npad, 3], F32, kind="ExternalInput")
    g_d = nc.dram_tensor("g", [128, NCOLS], F32, kind="ExternalInput")
    rz_d = nc.dram_tensor("rzb", [128, 3], F32, kind="ExternalInput")
    out_d = nc.dram_tensor("out", [npad, OUT_DIM], BF16, kind="ExternalOutput")

    with tile.TileContext(nc) as tc:
        with (
            tc.tile_pool(name="const", bufs=1) as constp,
            tc.tile_pool(name="wpool", bufs=4) as wpool,
            tc.tile_pool(name="opool", bufs=4) as opool,
            tc.tile_pool(name="mmps", bufs=2, space="PSUM") as mmpsp,
            tc.tile_pool(name="ftps", bufs=2, space="PSUM") as ftpsp,
        ):
            # ---- persistent tiles ----
            x_sb = constp.tile([128, ntiles, 3], F32)       # grouped point layout
            xb_sb = constp.tile([128, ntiles, 3], F32)      # rotated [x1,x2,x0]
            s_all = constp.tile([128, ntiles], F32)
            ang = constp.tile([128, ntiles, 4], F32)
            scr = constp.tile([128, ntiles], F32)
            trig = constp.tile([128, ntiles, 4], F32)
            q32 = constp.tile([128, ntiles, 6], F32)
            qh32 = constp.tile([128, ntiles, 6], F32)
            xh32 = constp.tile([128, ntiles, 3], F32)
            f_all = constp.tile([128, ntiles, KS], BF16)
            ft_all = constp.tile([128, ngrp, 128], BF16)
            g32_sb = constp.tile([128, NCOLS], F32)
            g_sb = constp.tile([128, NCOLS], BF16)
            rz_sb = constp.tile([128, 3], F32)
            ident = constp.tile([128, 128], F32)
            ident_bf = constp.tile([128, 128], BF16)

            nc.sync.dma_start(rz_sb[:], rz_d[:])
            make_identity(nc, ident[:])
            nc.gpsimd.tensor_copy(ident_bf[:], ident[:])
            bias_hpi = constp.tile([128, 1], F32)
            nc.gpsimd.memset(bias_hpi[:], HALF_PI)
            nc.gpsimd.memset(f_all[:, :, 27:28], 1.0)  # const feature
            ncf = GRP * KS  # 96 transposed columns per group

            # x load: partition p holds points p*ntiles .. p*ntiles+ntiles-1;
            # first 24 tiles land in their own DMA so chunk 0 starts early
            x_r = x_d[:].rearrange("(p m) c -> p m c", p=128)
            nc.sync.dma_start(x_sb[:, 0:24], x_r[:, 0:24])
            nc.sync.dma_start(g32_sb[:], g_d[:])
            nc.gpsimd.tensor_copy(g_sb[:], g32_sb[:])
            nc.sync.dma_start(x_sb[:, 24:ntiles], x_r[:, 24:ntiles])

            # ---- prologue (chunked so the main loop can start early) ----
            chunk_bounds = [0, 24, 108, ntiles]  # small first chunk: fast start
            fams = [
                (INV_2PI, 0.0, -TWO_PI, C1),     # sin(s)
                (INV_2PI, 0.25, -TWO_PI, C1C),   # sin(s + pi/2)
                (INV_PI, 0.0, -PI_F, C2),        # sin(2s)
                (INV_PI, 0.25, -PI_F, C2C),      # sin(2s + pi/2)
            ]
            def emit_prologue(cc):
                t0, t1 = chunk_bounds[cc], chunk_bounds[cc + 1]
                # s = x . rz  (tensor_scalar: scalar slot is per-partition)
                if cc == 0:
                    nc.vector.tensor_tensor(
                        scr[:, 0:3], x_sb[:, 0, :], rz_sb[:, 0:3], OP.mult
                    )
                nc.vector.tensor_scalar(
                    s_all[:, t0:t1], x_sb[:, t0:t1, 0], rz_sb[:, 0:1],
                    None, OP.mult
                )
                nc.vector.scalar_tensor_tensor(
                    s_all[:, t0:t1], x_sb[:, t0:t1, 1], rz_sb[:, 1:2],
                    s_all[:, t0:t1], OP.mult, OP.add
                )
                nc.vector.scalar_tensor_tensor(
                    s_all[:, t0:t1], x_sb[:, t0:t1, 2], rz_sb[:, 2:3],
                    s_all[:, t0:t1], OP.mult, OP.add
                )
                # range-reduce the four angle families into [-pi, pi] after
                # the activation's own scale/bias is applied; n = round(...)
                # via the 1.5*2^23 magic constant (delta added pre-magic)
                for ci, (inv, delta, mul, (lo, hi)) in enumerate(fams):
                    nc.vector.tensor_scalar(
                        scr[:, t0:t1], s_all[:, t0:t1], inv, delta,
                        OP.mult, OP.add
                    )
                    nc.vector.tensor_scalar(
                        scr[:, t0:t1], scr[:, t0:t1], MAGIC, MAGIC,
                        OP.add, OP.subtract
                    )
                    nc.vector.scalar_tensor_tensor(
                        scr[:, t0:t1], scr[:, t0:t1], mul, s_all[:, t0:t1],
                        OP.mult, OP.add
                    )
                    nc.vector.tensor_scalar(
                        ang[:, t0:t1, ci], scr[:, t0:t1], hi, lo,
                        OP.min, OP.max
                    )
                # ---- F assembly for this chunk ----
                nc.vector.tensor_copy(
                    xb_sb[:, t0:t1, 0:2], x_sb[:, t0:t1, 1:3]
                )
                nc.vector.tensor_copy(
                    xb_sb[:, t0:t1, 2:3], x_sb[:, t0:t1, 0:1]
                )
                nc.vector.tensor_tensor(
                    q32[:, t0:t1, :3], x_sb[:, t0:t1], x_sb[:, t0:t1], OP.mult
                )
                nc.vector.tensor_tensor(
                    q32[:, t0:t1, 3:], x_sb[:, t0:t1], xb_sb[:, t0:t1], OP.mult
                )
                # qh = bf16(q) happens in the f32->bf16 copy; ql = q - qh
                nc.vector.tensor_copy(f_all[:, t0:t1, 0:6], q32[:, t0:t1])
                nc.vector.tensor_copy(qh32[:, t0:t1], f_all[:, t0:t1, 0:6])
                nc.vector.tensor_tensor(
                    f_all[:, t0:t1, 6:12], q32[:, t0:t1], qh32[:, t0:t1],
                    OP.subtract
                )
                nc.gpsimd.tensor_copy(
                    f_all[:, t0:t1, 12:18], f_all[:, t0:t1, 0:6]
                )
                # x-chain + trig copy on the idle Pool engine (SBUF-only)
                nc.gpsimd.tensor_copy(f_all[:, t0:t1, 18:21], x_sb[:, t0:t1])
                nc.gpsimd.tensor_copy(xh32[:, t0:t1], f_all[:, t0:t1, 18:21])
                nc.gpsimd.tensor_tensor(
                    f_all[:, t0:t1, 21:24], x_sb[:, t0:t1], xh32[:, t0:t1],
                    OP.subtract
                )
                nc.gpsimd.tensor_copy(
                    f_all[:, t0:t1, 24:27], f_all[:, t0:t1, 18:21]
                )

            def emit_sins(cc):
                t0, t1 = chunk_bounds[cc], chunk_bounds[cc + 1]
                nc.scalar.activation(
                    trig[:, t0:t1, 0], ang[:, t0:t1, 0], AF.Sin
                )
                nc.scalar.activation(
                    trig[:, t0:t1, 1], ang[:, t0:t1, 1], AF.Sin,
                    bias=bias_hpi[:]
                )
                nc.scalar.activation(
                    trig[:, t0:t1, 2], ang[:, t0:t1, 2], AF.Sin, scale=2.0
                )
                nc.scalar.activation(
                    trig[:, t0:t1, 3], ang[:, t0:t1, 3], AF.Sin,
                    bias=bias_hpi[:], scale=2.0
                )
                nc.gpsimd.tensor_copy(f_all[:, t0:t1, 28:32], trig[:, t0:t1])

            # ---- main loop: per 3-tile group ----
            def emit_transpose(g):
                ft_ps = ftpsp.tile([128, 128], BF16, tag="FT")
                nc.tensor.matmul(
                    ft_ps[0:ncf, 0:128],
                    f_all[:, g * GRP : (g + 1) * GRP, :],
                    ident_bf[:],
                    is_transpose=True,
                )
                nc.scalar.copy(ft_all[0:ncf, g, :], ft_ps[0:ncf, :])

            emit_prologue(0)
            emit_sins(0)
            emit_prologue(1)
            emit_prologue(2)
            emit_transpose(0)
            for g in range(ngrp):
                if g + 1 < ngrp:
                    emit_transpose(g + 1)

                out_rows = out_d[:].rearrange("(p m) c -> p (m c)", p=128)[
                    :, g * GRP * OUT_DIM : (g + 1) * GRP * OUT_DIM
                ]
                o_t = opool.tile([128, GRP, 5, L], BF16, tag="O")
                psum = mmpsp.tile([128, GRP, 512], F32, tag="P")
                for j in range(GRP):
                    m = j * KS
                    nc.tensor.matmul(
                        psum[:, j, 0:NCOLS],
                        ft_all[m : m + K, g, :],
                        g_sb[m : m + K, :],
                        start=True,
                        stop=True,
                    )
                w_t = wpool.tile([128, GRP, L], F32, tag="W")
                nc.scalar.activation(
                    w_t[:], psum[:, :, 0:L], AF.Exp, scale=-0.5
                )
                enc5 = psum[:, :, L:NCOLS].rearrange("p t (b l) -> p t b l", l=L)
                wb5 = _bcast_block(w_t[:], 5, L)
                nc.vector.tensor_tensor(o_t[:], enc5, wb5, OP.mult)
                nc.sync.dma_start(
                    out_rows[:], o_t[:].rearrange("p t b l -> p (t b l)")
                )
                if g == 0:
                    # ACT is in-order: chunk 1/2 sins go behind exp(0) so the
                    # first weighted multiplies are never blocked
                    emit_sins(1)
                    emit_sins(2)
    nc.compile()
    return nc


def host_params(l, r, scaling, rotation):
    """G [128, 408] f32 (bf16-representable, replicated at 3 row-group
    bases) + rz broadcast. Feature rows per 32-block:
      0-5   qh  * Gh(quad)      6-11  ql * Gh(quad)   12-17 qh * Gl(quad)
      18-20 xh  * bh | rz*eta   21-23 xl * bh | rz*eta
      24-26 xh  * bl            27    1  * ch | -t*eta
      28-31 trig * (per-landmark trig * eta)
    where eta_k = exp(-0.5 * cl_k) folds the bf16 residual of c into the
    five enc blocks."""
    l = l.astype(np.float64)
    r = r.astype(np.float64)
    scaling = scaling.astype(np.float64)
    rotation = rotation.astype(np.float64)

    rz = r[:3, 2]
    qn = rotation / np.maximum(
        np.linalg.norm(rotation, axis=1, keepdims=True), 1e-12
    )
    w, x, y, z = qn[:, 0], qn[:, 1], qn[:, 2], qn[:, 3]
    R = np.empty((L, 3, 3), np.float64)
    R[:, 0, 0] = 1 - 2 * (y * y + z * z)
    R[:, 0, 1] = 2 * (x * y - w * z)
    R[:, 0, 2] = 2 * (x * z + w * y)
    R[:, 1, 0] = 2 * (x * y + w * z)
    R[:, 1, 1] = 1 - 2 * (x * x + z * z)
    R[:, 1, 2] = 2 * (y * z - w * x)
    R[:, 2, 0] = 2 * (x * z - w * y)
    R[:, 2, 1] = 2 * (y * z + w * x)
    R[:, 2, 2] = 1 - 2 * (x * x + y * y)
    M = R / scaling[:, None, :]
    cov = np.einsum("lij,lkj->lik", M, M)       # [L,3,3]

    b = np.einsum("lij,lj->li", cov, l)         # cov_k @ l_k
    c = np.einsum("li,li->l", l, b)             # l^T cov l
    t = l @ rz

    # quad rows matching features [x0^2, x1^2, x2^2, x0x1, x1x2, x2x0]
    Gq = np.stack(
        [
            cov[:, 0, 0],
            cov[:, 1, 1],
            cov[:, 2, 2],
            2 * cov[:, 0, 1],
            2 * cov[:, 1, 2],
            2 * cov[:, 0, 2],
        ]
    )                                            # [6, L]
    Gq_h, Gq_l = bf16_split(Gq)
    b2 = -2.0 * b.T                              # [3, L]
    b2_h, b2_l = bf16_split(b2)
    c_h, c_l = bf16_split(c)
    eta = np.exp(-0.5 * c_l)                     # bounded: |c_l| <= |c|*2^-8

    c1, s1 = np.cos(t), np.sin(t)
    c2, s2 = np.cos(2 * t), np.sin(2 * t)

    G = np.zeros((K, NCOLS), np.float64)
    # wq block
    G[0:6, 0:L] = Gq_h
    G[6:12, 0:L] = Gq_h
    G[12:18, 0:L] = Gq_l
    G[18:21, 0:L] = b2_h
    G[21:24, 0:L] = b2_h
    G[24:27, 0:L] = b2_l
    G[27, 0:L] = c_h
    # dz block (xh + xl recovers full-precision x)
    G[18:21, L : 2 * L] = rz[:, None] * eta[None, :]
    G[21:24, L : 2 * L] = rz[:, None] * eta[None, :]
    G[27, L : 2 * L] = -t * eta
    # trig blocks: sin(s-t) = sinS cosT - cosS sinT ; cos(s-t) = ...
    G[28, 2 * L : 3 * L] = c1 * eta
    G[29, 2 * L : 3 * L] = -s1 * eta
    G[28, 3 * L : 4 * L] = s1 * eta
    G[29, 3 * L : 4 * L] = c1 * eta
    G[30, 4 * L : 5 * L] = c2 * eta
    G[31, 4 * L : 5 * L] = -s2 * eta
    G[30, 5 * L : 6 * L] = s2 * eta
    G[31, 5 * L : 6 * L] = c2 * eta
    # pre-round everything to bf16-representable f32 so the host splits
    # are exactly what the PE multiplies
    Gbf, _ = bf16_split(G)
    Grep = np.zeros((128, NCOLS), np.float32)
    for m in range(GRP):
        Grep[m * KS : m * KS + K, :] = Gbf.astype(np.float32)
    return Grep, np.broadcast_to(
        rz.astype(np.float32), (128, 3)
    ).copy()


_NC_CACHE = {}


def _get_nc():
    if "nc" not in _NC_CACHE:
        _NC_CACHE["nc"] = build_nc()
    return _NC_CACHE["nc"]


def run(inputs, mm_f32r=True, trace=False):
    x = inputs["x"]
    G, rzb = host_params(
        inputs["l"], inputs["r"], inputs["scaling"], inputs["rotation"]
    )
    xpad = np.zeros((NCORES * NPAD, 3), np.float32)
    xpad[:N] = x
    shards = xpad.reshape(NCORES, NPAD, 3)
    in_maps = [
        {"x": np.ascontiguousarray(shards[i]), "g": G, "rzb": rzb}
        for i in range(NCORES)
    ]
    nc = _get_nc()
    res = run_bass_kernel_spmd(nc, in_maps, list(range(NCORES)), trace=trace)
    out = np.concatenate([r["out"] for r in res.results], axis=0)[:N]
    return out.astype(np.float32), res


def kernel(**inputs):
    out, _ = run(inputs)
    return out
